# revision 35
# baseline (speedup 1.0000x reference)
"""Trainium2 Bass kernel for nn_AttentiveGatingv2 (moe_routing).

Reference computation (shapes hardcoded):
  x: [64, 96, 207, 64] -> take last 6 timesteps -> per-(b,n) token:
  z = proj(x_k); qkv = in_proj(z); 4-head attention over the 6 steps;
  out-proj; mean over steps; fc to 8 experts; softmax -> [64, 207, 8].

Host-side algebraic fusion (verified vs reference):
  W_eff = in_proj_w @ proj_w  (96x64), b_eff = in_proj_w@proj_b + in_proj_b
  (q-rows pre-scaled by 1/sqrt(8)); since mean-over-steps commutes with the
  linear out-proj/fc, post-attention collapses to
  logits = G @ (sum_j wbar_j * v_j) + g_b  with  G = fc_w@out_w/6,
  g_b = fc_w@out_b + fc_b,  wbar_j = sum_i softmax_j(scores)_ij.

Layout strategy: host pre-slices the 6 needed timesteps (1/16 of x), casts
to bf16, packs them feature-major with an appended ones-row so the single
PE matmul  qkv[tokens,96] = x_aug[65,tokens].T @ W_aug[65,96]  lands
token-major in PSUM (fp32) with bias included.  Attention math runs on
VectorE in bf16 (validated ~5e-4 rel-to-max on the final softmax output),
with 128-token tiles processed in pairs so the small softmax/context ops
amortize instruction overhead; the paired expert-logit matmul uses a
block-diagonal G.  8 NeuronCores data-parallel over batch; no comms.
"""

import numpy as np
import ml_dtypes

import concourse.bass as bass
import concourse.mybir as mybir
import concourse.tile as tile
from concourse.bacc import Bacc
from concourse.bass_utils import run_bass_kernel_spmd

F32 = mybir.dt.float32
BF16 = mybir.dt.bfloat16
NP_BF16 = ml_dtypes.bfloat16

# problem dims
B, T, NTOK, C = 64, 96, 207, 64
D, H, HD, K = 32, 4, 8, 6
E = 8
NCORES = 8

# per-core dims
B_SH = B // NCORES            # 8
S = B_SH * NTOK               # 1656 tokens per core
P = 128
NT = (S + P - 1) // P         # 13 tiles
S_PAD = NT * P                # 1664
CA = C + 1                    # 65: channels + ones row
E3 = 3 * D                    # 96
KK = K * K                    # 36
DA = D + 1                    # 33


def _build_module():
    nc = Bacc()

    xt = nc.dram_tensor("xt", [CA, K, S_PAD], BF16, kind="ExternalInput")
    wa = nc.dram_tensor("wa", [CA, E3], BF16, kind="ExternalInput")
    # cf packs block-diag G2_aug [66,16] | identity [128,128] (fp32)
    cf = nc.dram_tensor("cf", [P, 2 * E + P], F32, kind="ExternalInput")
    # out[p, t, e]: token (t*128+p); host reassembles. This layout keeps the
    # single final store one-descriptor-per-partition contiguous.
    out = nc.dram_tensor("out", [P, NT, E], F32, kind="ExternalOutput")

    AF = mybir.ActivationFunctionType
    AX = mybir.AxisListType

    def apv(t, dims, extra_offset=0):
        # custom AP over tile t: keep t's partition dim, replace free dims
        return bass.AP(
            tensor=t.tensor,
            offset=t.offset + extra_offset,
            ap=[list(t.ap[0])] + [list(d) for d in dims],
        )

    groups = [(t, min(2, NT - t)) for t in range(0, NT, 2)]  # (base, size)

    with tile.TileContext(nc) as tc:
        with (
            tc.tile_pool(name="singles", bufs=1) as singles,
            tc.tile_pool(name="xload", bufs=NT) as xload,
            tc.tile_pool(name="work", bufs=2) as work,
            tc.tile_pool(name="psum", bufs=2, space="PSUM") as psum,
        ):
            # tile-0 load first so its data isn't serialized behind the
            # constants on the DMA issue path
            xt_tiles = []
            xt_dmas = []
            for t in range(NT):
                xt_sb = xload.tile([CA, K, P], BF16, name=f"xt_sb{t}",
                                   tag="xt")
                xd = nc.sync.dma_start(out=xt_sb,
                                       in_=xt[:, :, t * P:(t + 1) * P])
                xt_tiles.append(xt_sb)
                xt_dmas.append(xd)

            wa_sb = singles.tile([CA, E3], BF16)
            nc.sync.dma_start(out=wa_sb, in_=wa[:, :])
            cf_sb = singles.tile([P, 2 * E + P], F32)
            nc.sync.dma_start(out=cf_sb, in_=cf[:, :])
            ga2_sb = cf_sb[0:2 * DA, 0:2 * E]
            id_sb = cf_sb[:, 2 * E:]

            out_sb = singles.tile([P, NT, E], F32)

            qkv_first = []
            for tg, g in groups:
                qk_sb = work.tile([P, 2, K, E3], BF16)
                tmp = work.tile([P, 2, KK, D], BF16)
                for u in range(g):
                    t = tg + u
                    xt_sb = xt_tiles[t]
                    if t >= 4:
                        # stagger loads ~4 tiles ahead of compute so tile 0
                        # data doesn't round-robin behind all 13 loads
                        tile.add_dep_helper(xt_dmas[t].ins,
                                            qkv_first[t - 4].ins,
                                            sync=True, reason="load stagger")

                    # ---- qkv: 6 matmuls (bf16 in, fp32 psum) ----
                    # [P, 8, 128] = exactly 2 PSUM banks so slots are
                    # bank-aligned (1.5-bank slots would share a bank)
                    qkv_ps = psum.tile([P, 8, 128], F32, tag="qkv_ps", bufs=2,
                                       name="qkv_ps")
                    for i in range(K):
                        mm = nc.tensor.matmul(
                            out=qkv_ps[:, i, 0:E3],
                            lhsT=xt_sb[:, i, :],
                            rhs=wa_sb[:, :],
                            start=True,
                            stop=True,
                        )
                        if i == 0:
                            qkv_first.append(mm)

                    # ---- evacuate q,k,v to SBUF as bf16 on ScalarE ----
                    nc.scalar.copy(out=qk_sb[:, u], in_=qkv_ps[:, 0:K, 0:E3])

                    # ---- scores tmp[i,j,(h,c)] = q[i,(hc)] * k[j,(hc)] ----
                    # (TensorTensor allows at most 3 free AP dims: (h,c) is
                    # kept merged, and the i/j broadcasts force per-tile muls)
                    off = u * K * E3
                    q_ap = apv(qk_sb, [[E3, K], [0, K], [1, D]], off)
                    k_ap = apv(qk_sb, [[0, K], [E3, K], [1, D]], off + D)
                    tm_out = apv(tmp, [[D, KK], [1, D]], u * KK * D)
                    nc.vector.tensor_mul(tm_out, q_ap, k_ap)

                # ---- batched over the pair from here on ----
                gKK = g * KK
                # scores[(t,i,j), h] = sum_c tmp  (tensor_reduce is 1x but a
                # bf16 add-tree measured no better: short strided runs don't
                # engage the 2x mode)
                scores = work.tile([P, 2, KK, H], F32)
                in_ap = apv(tmp, [[D, gKK], [HD, H], [1, HD]])
                o_ap = apv(scores, [[H, gKK], [1, H]])
                nc.vector.reduce_sum(out=o_ap, in_=in_ap, axis=AX.X)

                # ---- softmax over j (scores pre-scaled, |s|<1.5) ----
                es = work.tile([P, 2, K, K, H], BF16)
                nc.scalar.activation(out=es[:, 0:g], in_=scores[:, 0:g],
                                     func=AF.Exp)
                zs = work.tile([P, 2, K, H], F32)
                es_jred = apv(es, [[K * H, g * K], [1, H], [H, K]])  # [(t,i),h,j]
                zs_o = apv(zs, [[1, g * K * H]])
                nc.vector.reduce_sum(out=zs_o, in_=es_jred, axis=AX.X)
                rs = work.tile([P, 2, K, H], F32)
                nc.vector.reciprocal(rs[:, 0:g], zs[:, 0:g])
                # attn[(t,i),j,h] = es * rs
                attn = work.tile([P, 2, K, K, H], BF16)
                es_ap = apv(es, [[K * H, g * K], [H, K], [1, H]])
                rs_ap = apv(rs, [[H, g * K], [0, K], [1, H]])
                at_o = apv(attn, [[K * H, g * K], [H, K], [1, H]])
                nc.vector.tensor_mul(at_o, es_ap, rs_ap)

                # ---- wbar[t,(j,h)] = sum_i attn[t,i,(jh)] ----
                wbar = work.tile([P, 2, K, H], F32)
                at_ap = apv(attn, [[K * K * H, g], [1, K * H], [K * H, K]])
                wb_o = apv(wbar, [[K * H, g], [1, K * H]])
                nc.vector.reduce_sum(out=wb_o, in_=at_ap, axis=AX.X)

                # ---- ct[(t,j),h,c] = wbar[(t,jh)] * v[t,j,(hc)] ----
                ct = work.tile([P, 2, K, H, HD], BF16)
                wb_ap = apv(wbar, [[1, g * K * H], [0, HD]])
                v_ap = apv(qk_sb, [[K * E3, g], [E3, K], [1, D]], 2 * D)
                ct_o = apv(ct, [[1, g * K * D]])
                nc.vector.tensor_mul(ct_o, wb_ap, v_ap)

                # ---- ctxbar[t,(h,c)] = sum_j ct; col D set to 1 ----
                cb = work.tile([P, 2, DA], F32)
                ct_ap = apv(ct, [[K * D, g], [1, D], [D, K]])  # [t,(hc),j]
                cb_o = apv(cb, [[DA, g], [1, D]])
                nc.vector.reduce_sum(out=cb_o, in_=ct_ap, axis=AX.X)
                nc.vector.memset(cb[:, 0:g, D:DA], 1.0)

                # ---- logits: one transpose + one block-diag matmul ----
                ctT_ps = psum.tile([2 * DA, P], F32, bufs=2, name="ctT_ps")
                nc.tensor.transpose(ctT_ps[0:g * DA, :],
                                    cb[:, 0:g, :], id_sb)
                ctT_sb = work.tile([2 * DA, P], F32)
                nc.scalar.copy(out=ctT_sb[0:g * DA], in_=ctT_ps[0:g * DA])

                log_ps = psum.tile([P, 2 * E], F32, bufs=2, name="log_ps")
                nc.tensor.matmul(
                    out=log_ps[:, 0:g * E],
                    lhsT=ctT_sb[0:g * DA, :],
                    rhs=ga2_sb[0:g * DA, 0:g * E],
                    start=True, stop=True,
                )

                # ---- final softmax over 8 experts (batched) ----
                el = work.tile([P, 2, E], F32)
                nc.scalar.activation(out=el[:, 0:g], in_=log_ps[:, 0:g * E],
                                     func=AF.Exp)
                zf = work.tile([P, 2], F32)
                nc.vector.reduce_sum(out=zf[:, 0:g], in_=el[:, 0:g],
                                     axis=AX.X)
                rf = work.tile([P, 2], F32)
                nc.vector.reciprocal(rf[:, 0:g], zf[:, 0:g])
                rf_ap = apv(rf, [[1, g], [0, E]])
                nc.vector.tensor_mul(out_sb[:, tg:tg + g, :],
                                     el[:, 0:g], rf_ap)

            # single output store on the software-DGE path (one sync wait)
            nc.gpsimd.dma_start(out=out[:, :, :], in_=out_sb)

    nc.finalize()
    return nc


_NC = None


def _get_module():
    global _NC
    if _NC is None:
        _NC = _build_module()
    return _NC


def _host_prep(x, proj_w, proj_b, in_proj_w, in_proj_b, out_w, out_b, fc_w, fc_b):
    scale = np.float32(1.0 / np.sqrt(HD))
    w_eff = (in_proj_w @ proj_w).astype(np.float32)          # [96, 64]
    b_eff = (in_proj_w @ proj_b + in_proj_b).astype(np.float32)
    w_eff[0:D] *= scale
    b_eff[0:D] *= scale
    wa = np.concatenate([w_eff.T, b_eff[None, :]], axis=0)   # [65, 96]
    wa = np.ascontiguousarray(wa).astype(NP_BF16)

    g = (fc_w @ out_w / np.float32(K)).astype(np.float32)    # [8, 32]
    g_b = (fc_w @ out_b + fc_b).astype(np.float32)
    ga = np.concatenate([g.T, g_b[None, :]], axis=0)         # [33, 8]

    cf = np.zeros((P, 2 * E + P), dtype=np.float32)
    cf[0:DA, 0:E] = ga
    cf[DA:2 * DA, E:2 * E] = ga                               # block-diag
    cf[:, 2 * E:] = np.eye(P, dtype=np.float32)

    # x: [B, T, N, C] -> last K steps -> per-core [65, K, S_PAD] feature-major
    xk = x[:, T - K:, :, :]                                  # [B, K, N, C]
    in_maps = []
    for core in range(NCORES):
        xc = xk[core * B_SH:(core + 1) * B_SH]               # [8, K, N, C]
        # -> [C, K, b, N] -> [C, K, S]
        xc = np.transpose(xc, (3, 1, 0, 2)).reshape(C, K, S)
        xtc = np.ones((CA, K, S_PAD), dtype=NP_BF16)
        xtc[0:C, :, 0:S] = xc.astype(NP_BF16)
        xtc[0:C, :, S:] = 0
        in_maps.append({"xt": xtc, "wa": wa, "cf": cf})
    return in_maps


def kernel(x, proj_w, proj_b, in_proj_w, in_proj_b, out_w, out_b, fc_w, fc_b,
           _trace=False):
    in_maps = _host_prep(np.asarray(x, dtype=np.float32),
                         np.asarray(proj_w, dtype=np.float32),
                         np.asarray(proj_b, dtype=np.float32),
                         np.asarray(in_proj_w, dtype=np.float32),
                         np.asarray(in_proj_b, dtype=np.float32),
                         np.asarray(out_w, dtype=np.float32),
                         np.asarray(out_b, dtype=np.float32),
                         np.asarray(fc_w, dtype=np.float32),
                         np.asarray(fc_b, dtype=np.float32))
    nc = _get_module()
    res = run_bass_kernel_spmd(nc, in_maps, core_ids=list(range(NCORES)),
                               trace=_trace)
    outs = []
    for core in range(NCORES):
        oc = res.results[core]["out"]                        # [P, NT, E]
        oc = oc.transpose(1, 0, 2).reshape(S_PAD, E)[:S]
        oc = oc.reshape(B_SH, NTOK, E)
        outs.append(oc)
    full = np.concatenate(outs, axis=0)                      # [64, 207, 8]
    if _trace:
        kernel._last_exec_time_ns = res.exec_time_ns
        kernel._last_profile = res.profile_json
    return full.astype(np.float32)


# revision 36
# speedup vs baseline: 1.1166x; 1.1166x over previous
"""Trainium2 Bass kernel for nn_AttentiveGatingv2 (moe_routing).

Reference computation (shapes hardcoded):
  x: [64, 96, 207, 64] -> take last 6 timesteps -> per-(b,n) token:
  z = proj(x_k); qkv = in_proj(z); 4-head attention over the 6 steps;
  out-proj; mean over steps; fc to 8 experts; softmax -> [64, 207, 8].

Host-side algebraic fusion (verified vs reference):
  W_eff = in_proj_w @ proj_w  (96x64), b_eff = in_proj_w@proj_b + in_proj_b
  (q-rows pre-scaled by 1/sqrt(8)); since mean-over-steps commutes with the
  linear out-proj/fc, post-attention collapses to
  logits = G @ (sum_j wbar_j * v_j) + g_b  with  G = fc_w@out_w/6,
  g_b = fc_w@out_b + fc_b,  wbar_j = sum_i softmax_j(scores)_ij.

Layout strategy: host pre-slices the 6 needed timesteps (1/16 of x), casts
to bf16, packs them feature-major with an appended ones-row so the single
PE matmul  qkv[tokens,96] = x_aug[65,tokens].T @ W_aug[65,96]  lands
token-major in PSUM (fp32) with bias included.  Attention math runs on
VectorE in bf16 (validated ~5e-4 rel-to-max on the final softmax output),
with 128-token tiles processed in pairs so the small softmax/context ops
amortize instruction overhead; the paired expert-logit matmul uses a
block-diagonal G.  8 NeuronCores data-parallel over batch; no comms.
"""

import numpy as np
import ml_dtypes

import concourse.bass as bass
import concourse.mybir as mybir
import concourse.tile as tile
from concourse.bacc import Bacc
from concourse.bass_utils import run_bass_kernel_spmd

F32 = mybir.dt.float32
BF16 = mybir.dt.bfloat16
NP_BF16 = ml_dtypes.bfloat16

# problem dims
B, T, NTOK, C = 64, 96, 207, 64
D, H, HD, K = 32, 4, 8, 6
E = 8
NCORES = 8

# per-core dims
B_SH = B // NCORES            # 8
S = B_SH * NTOK               # 1656 tokens per core
P = 128
NT = (S + P - 1) // P         # 13 tiles
S_PAD = NT * P                # 1664
CA = C + 1                    # 65: channels + ones row
E3 = 3 * D                    # 96
KK = K * K                    # 36
DA = D + 1                    # 33


def _build_module():
    nc = Bacc()

    xt = nc.dram_tensor("xt", [CA, K, S_PAD], BF16, kind="ExternalInput")
    wa = nc.dram_tensor("wa", [CA, E3], BF16, kind="ExternalInput")
    # cf packs block-diag G2_aug [66,16] | identity [128,128] (fp32)
    cf = nc.dram_tensor("cf", [P, 2 * E + P], F32, kind="ExternalInput")
    # out[p, t, e]: token (t*128+p); host reassembles. This layout keeps the
    # single final store one-descriptor-per-partition contiguous.
    out = nc.dram_tensor("out", [P, NT, E], F32, kind="ExternalOutput")

    AF = mybir.ActivationFunctionType
    AX = mybir.AxisListType

    def apv(t, dims, extra_offset=0):
        # custom AP over tile t: keep t's partition dim, replace free dims
        return bass.AP(
            tensor=t.tensor,
            offset=t.offset + extra_offset,
            ap=[list(t.ap[0])] + [list(d) for d in dims],
        )

    groups = [(t, min(2, NT - t)) for t in range(0, NT, 2)]  # (base, size)

    with tile.TileContext(nc) as tc:
        with (
            tc.tile_pool(name="singles", bufs=1) as singles,
            tc.tile_pool(name="xload", bufs=NT) as xload,
            tc.tile_pool(name="work", bufs=2) as work,
            tc.tile_pool(name="psum", bufs=2, space="PSUM") as psum,
        ):
            # tile-0 load first so its data isn't serialized behind the
            # constants on the DMA issue path
            xt_tiles = []
            xt_dmas = []
            for t in range(NT):
                xt_sb = xload.tile([CA, K, P], BF16, name=f"xt_sb{t}",
                                   tag="xt")
                xd = nc.sync.dma_start(out=xt_sb,
                                       in_=xt[:, :, t * P:(t + 1) * P])
                xt_tiles.append(xt_sb)
                xt_dmas.append(xd)

            wa_sb = singles.tile([CA, E3], BF16)
            nc.sync.dma_start(out=wa_sb, in_=wa[:, :])
            cf_sb = singles.tile([P, 2 * E + P], F32)
            nc.sync.dma_start(out=cf_sb, in_=cf[:, :])
            ga2_sb = cf_sb[0:2 * DA, 0:2 * E]
            id_sb = cf_sb[:, 2 * E:]

            out_sb = singles.tile([P, NT, E], F32)

            qkv_first = []
            for tg, g in groups:
                qk_sb = work.tile([P, 2, K, E3], BF16)
                tmp = work.tile([P, 2, KK, D], BF16)
                for u in range(g):
                    t = tg + u
                    xt_sb = xt_tiles[t]
                    if t >= 4:
                        # stagger loads ~4 tiles ahead of compute so tile 0
                        # data doesn't round-robin behind all 13 loads
                        tile.add_dep_helper(xt_dmas[t].ins,
                                            qkv_first[t - 4].ins,
                                            sync=True, reason="load stagger")

                    # ---- qkv: 6 matmuls (bf16 in, fp32 psum) ----
                    # [P, 8, 128] = exactly 2 PSUM banks so slots are
                    # bank-aligned (1.5-bank slots would share a bank)
                    qkv_ps = psum.tile([P, 8, 128], F32, tag="qkv_ps", bufs=2,
                                       name="qkv_ps")
                    for i in range(K):
                        mm = nc.tensor.matmul(
                            out=qkv_ps[:, i, 0:E3],
                            lhsT=xt_sb[:, i, :],
                            rhs=wa_sb[:, :],
                            start=True,
                            stop=True,
                        )
                        if i == 0:
                            qkv_first.append(mm)

                    # ---- evacuate q,k,v to SBUF as bf16 on ScalarE ----
                    nc.scalar.copy(out=qk_sb[:, u], in_=qkv_ps[:, 0:K, 0:E3])

                    # ---- scores tmp[i,j,(h,c)] = q[i,(hc)] * k[j,(hc)] ----
                    # (TensorTensor allows at most 3 free AP dims: (h,c) is
                    # kept merged, and the i/j broadcasts force per-tile muls)
                    off = u * K * E3
                    q_ap = apv(qk_sb, [[E3, K], [0, K], [1, D]], off)
                    k_ap = apv(qk_sb, [[0, K], [E3, K], [1, D]], off + D)
                    tm_out = apv(tmp, [[D, KK], [1, D]], u * KK * D)
                    nc.vector.tensor_mul(tm_out, q_ap, k_ap)

                # ---- batched over the pair from here on ----
                gKK = g * KK
                # scores[(t,i,j), h] = sum_c tmp via add tree: tensor_reduce
                # costs input-elems (2304/pair @1x) vs 1152+576+576 for the
                # tree (TT cost follows output elems)
                s1 = work.tile([P, 2, KK, H, 4], BF16)
                a_ap = apv(tmp, [[D, gKK], [HD, H], [1, 4]])
                b_ap = apv(tmp, [[D, gKK], [HD, H], [1, 4]], 4)
                o_ap = apv(s1, [[16, gKK], [4, H], [1, 4]])
                nc.vector.tensor_add(o_ap, a_ap, b_ap)
                s2 = work.tile([P, 2, KK, H, 2], BF16)
                a_ap = apv(s1, [[16, gKK], [4, H], [1, 2]])
                b_ap = apv(s1, [[16, gKK], [4, H], [1, 2]], 2)
                o_ap = apv(s2, [[8, gKK], [2, H], [1, 2]])
                nc.vector.tensor_add(o_ap, a_ap, b_ap)
                scores = work.tile([P, 2, KK, H], F32)
                a_ap = apv(s2, [[8, gKK], [2, H]])
                b_ap = apv(s2, [[8, gKK], [2, H]], 1)
                o_ap = apv(scores, [[H, gKK], [1, H]])
                nc.vector.tensor_add(o_ap, a_ap, b_ap)

                # ---- softmax over j (scores pre-scaled, |s|<1.5) ----
                es = work.tile([P, 2, K, K, H], BF16)
                nc.scalar.activation(out=es[:, 0:g], in_=scores[:, 0:g],
                                     func=AF.Exp)
                zs = work.tile([P, 2, K, H], F32)
                es_jred = apv(es, [[K * H, g * K], [1, H], [H, K]])  # [(t,i),h,j]
                zs_o = apv(zs, [[1, g * K * H]])
                nc.vector.reduce_sum(out=zs_o, in_=es_jred, axis=AX.X)
                rs = work.tile([P, 2, K, H], F32)
                nc.vector.reciprocal(rs[:, 0:g], zs[:, 0:g])
                # attn[(t,i),j,h] = es * rs
                attn = work.tile([P, 2, K, K, H], BF16)
                es_ap = apv(es, [[K * H, g * K], [H, K], [1, H]])
                rs_ap = apv(rs, [[H, g * K], [0, K], [1, H]])
                at_o = apv(attn, [[K * H, g * K], [H, K], [1, H]])
                nc.vector.tensor_mul(at_o, es_ap, rs_ap)

                # ---- wbar[t,(j,h)] = sum_i attn[t,i,(jh)] ----
                wbar = work.tile([P, 2, K, H], F32)
                at_ap = apv(attn, [[K * K * H, g], [1, K * H], [K * H, K]])
                wb_o = apv(wbar, [[K * H, g], [1, K * H]])
                nc.vector.reduce_sum(out=wb_o, in_=at_ap, axis=AX.X)

                # ---- ct[(t,j),h,c] = wbar[(t,jh)] * v[t,j,(hc)] ----
                ct = work.tile([P, 2, K, H, HD], BF16)
                wb_ap = apv(wbar, [[1, g * K * H], [0, HD]])
                v_ap = apv(qk_sb, [[K * E3, g], [E3, K], [1, D]], 2 * D)
                ct_o = apv(ct, [[1, g * K * D]])
                nc.vector.tensor_mul(ct_o, wb_ap, v_ap)

                # ---- ctxbar[t,(h,c)] = sum_j ct; col D set to 1 ----
                cb = work.tile([P, 2, DA], F32)
                ct_ap = apv(ct, [[K * D, g], [1, D], [D, K]])  # [t,(hc),j]
                cb_o = apv(cb, [[DA, g], [1, D]])
                nc.vector.reduce_sum(out=cb_o, in_=ct_ap, axis=AX.X)
                nc.vector.memset(cb[:, 0:g, D:DA], 1.0)

                # ---- logits: one transpose + one block-diag matmul ----
                ctT_ps = psum.tile([2 * DA, P], F32, bufs=2, name="ctT_ps")
                nc.tensor.transpose(ctT_ps[0:g * DA, :],
                                    cb[:, 0:g, :], id_sb)
                ctT_sb = work.tile([2 * DA, P], F32)
                nc.scalar.copy(out=ctT_sb[0:g * DA], in_=ctT_ps[0:g * DA])

                log_ps = psum.tile([P, 2 * E], F32, bufs=2, name="log_ps")
                nc.tensor.matmul(
                    out=log_ps[:, 0:g * E],
                    lhsT=ctT_sb[0:g * DA, :],
                    rhs=ga2_sb[0:g * DA, 0:g * E],
                    start=True, stop=True,
                )

                # ---- final softmax over 8 experts (batched) ----
                el = work.tile([P, 2, E], F32)
                nc.scalar.activation(out=el[:, 0:g], in_=log_ps[:, 0:g * E],
                                     func=AF.Exp)
                zf = work.tile([P, 2], F32)
                nc.vector.reduce_sum(out=zf[:, 0:g], in_=el[:, 0:g],
                                     axis=AX.X)
                rf = work.tile([P, 2], F32)
                nc.vector.reciprocal(rf[:, 0:g], zf[:, 0:g])
                rf_ap = apv(rf, [[1, g], [0, E]])
                nc.vector.tensor_mul(out_sb[:, tg:tg + g, :],
                                     el[:, 0:g], rf_ap)

            # single output store on the software-DGE path (one sync wait)
            nc.gpsimd.dma_start(out=out[:, :, :], in_=out_sb)

    nc.finalize()
    return nc


_NC = None


def _get_module():
    global _NC
    if _NC is None:
        _NC = _build_module()
    return _NC


def _host_prep(x, proj_w, proj_b, in_proj_w, in_proj_b, out_w, out_b, fc_w, fc_b):
    scale = np.float32(1.0 / np.sqrt(HD))
    w_eff = (in_proj_w @ proj_w).astype(np.float32)          # [96, 64]
    b_eff = (in_proj_w @ proj_b + in_proj_b).astype(np.float32)
    w_eff[0:D] *= scale
    b_eff[0:D] *= scale
    wa = np.concatenate([w_eff.T, b_eff[None, :]], axis=0)   # [65, 96]
    wa = np.ascontiguousarray(wa).astype(NP_BF16)

    g = (fc_w @ out_w / np.float32(K)).astype(np.float32)    # [8, 32]
    g_b = (fc_w @ out_b + fc_b).astype(np.float32)
    ga = np.concatenate([g.T, g_b[None, :]], axis=0)         # [33, 8]

    cf = np.zeros((P, 2 * E + P), dtype=np.float32)
    cf[0:DA, 0:E] = ga
    cf[DA:2 * DA, E:2 * E] = ga                               # block-diag
    cf[:, 2 * E:] = np.eye(P, dtype=np.float32)

    # x: [B, T, N, C] -> last K steps -> per-core [65, K, S_PAD] feature-major
    xk = x[:, T - K:, :, :]                                  # [B, K, N, C]
    in_maps = []
    for core in range(NCORES):
        xc = xk[core * B_SH:(core + 1) * B_SH]               # [8, K, N, C]
        # -> [C, K, b, N] -> [C, K, S]
        xc = np.transpose(xc, (3, 1, 0, 2)).reshape(C, K, S)
        xtc = np.ones((CA, K, S_PAD), dtype=NP_BF16)
        xtc[0:C, :, 0:S] = xc.astype(NP_BF16)
        xtc[0:C, :, S:] = 0
        in_maps.append({"xt": xtc, "wa": wa, "cf": cf})
    return in_maps


def kernel(x, proj_w, proj_b, in_proj_w, in_proj_b, out_w, out_b, fc_w, fc_b,
           _trace=False):
    in_maps = _host_prep(np.asarray(x, dtype=np.float32),
                         np.asarray(proj_w, dtype=np.float32),
                         np.asarray(proj_b, dtype=np.float32),
                         np.asarray(in_proj_w, dtype=np.float32),
                         np.asarray(in_proj_b, dtype=np.float32),
                         np.asarray(out_w, dtype=np.float32),
                         np.asarray(out_b, dtype=np.float32),
                         np.asarray(fc_w, dtype=np.float32),
                         np.asarray(fc_b, dtype=np.float32))
    nc = _get_module()
    res = run_bass_kernel_spmd(nc, in_maps, core_ids=list(range(NCORES)),
                               trace=_trace)
    outs = []
    for core in range(NCORES):
        oc = res.results[core]["out"]                        # [P, NT, E]
        oc = oc.transpose(1, 0, 2).reshape(S_PAD, E)[:S]
        oc = oc.reshape(B_SH, NTOK, E)
        outs.append(oc)
    full = np.concatenate(outs, axis=0)                      # [64, 207, 8]
    if _trace:
        kernel._last_exec_time_ns = res.exec_time_ns
        kernel._last_profile = res.profile_json
    return full.astype(np.float32)


# revision 39
# speedup vs baseline: 1.1347x; 1.0162x over previous
"""Trainium2 Bass kernel for nn_AttentiveGatingv2 (moe_routing).

Reference computation (shapes hardcoded):
  x: [64, 96, 207, 64] -> take last 6 timesteps -> per-(b,n) token:
  z = proj(x_k); qkv = in_proj(z); 4-head attention over the 6 steps;
  out-proj; mean over steps; fc to 8 experts; softmax -> [64, 207, 8].

Host-side algebraic fusion (verified vs reference):
  W_eff = in_proj_w @ proj_w  (96x64), b_eff = in_proj_w@proj_b + in_proj_b
  (q-rows pre-scaled by 1/sqrt(8)); since mean-over-steps commutes with the
  linear out-proj/fc, post-attention collapses to
  logits = G @ (sum_j wbar_j * v_j) + g_b  with  G = fc_w@out_w/6,
  g_b = fc_w@out_b + fc_b,  wbar_j = sum_i softmax_j(scores)_ij.

Layout strategy: host pre-slices the 6 needed timesteps (1/16 of x), casts
to bf16, packs them feature-major with an appended ones-row so the single
PE matmul  qkv[tokens,96] = x_aug[65,tokens].T @ W_aug[65,96]  lands
token-major in PSUM (fp32) with bias included.  Attention math runs on
VectorE in bf16 (validated ~5e-4 rel-to-max on the final softmax output),
with 128-token tiles processed in pairs so the small softmax/context ops
amortize instruction overhead; the paired expert-logit matmul uses a
block-diagonal G.  8 NeuronCores data-parallel over batch; no comms.
"""

import numpy as np
import ml_dtypes

import concourse.bass as bass
import concourse.mybir as mybir
import concourse.tile as tile
from concourse.bacc import Bacc
from concourse.bass_utils import run_bass_kernel_spmd

F32 = mybir.dt.float32
BF16 = mybir.dt.bfloat16
NP_BF16 = ml_dtypes.bfloat16

# problem dims
B, T, NTOK, C = 64, 96, 207, 64
D, H, HD, K = 32, 4, 8, 6
E = 8
NCORES = 8

# per-core dims
B_SH = B // NCORES            # 8
S = B_SH * NTOK               # 1656 tokens per core
P = 128
NT = (S + P - 1) // P         # 13 tiles
S_PAD = NT * P                # 1664
CA = C + 1                    # 65: channels + ones row
E3 = 3 * D                    # 96
KK = K * K                    # 36
DA = D + 1                    # 33


def _build_module():
    nc = Bacc()

    xt = nc.dram_tensor("xt", [CA, K, S_PAD], BF16, kind="ExternalInput")
    wa = nc.dram_tensor("wa", [CA, E3], BF16, kind="ExternalInput")
    # cf packs block-diag G2_aug [66,16] | identity [128,128] (fp32)
    cf = nc.dram_tensor("cf", [P, 2 * E + P], F32, kind="ExternalInput")
    # out[p, t, e]: token (t*128+p); host reassembles. This layout keeps the
    # single final store one-descriptor-per-partition contiguous.
    out = nc.dram_tensor("out", [P, NT, E], F32, kind="ExternalOutput")

    AF = mybir.ActivationFunctionType
    AX = mybir.AxisListType

    def apv(t, dims, extra_offset=0):
        # custom AP over tile t: keep t's partition dim, replace free dims
        return bass.AP(
            tensor=t.tensor,
            offset=t.offset + extra_offset,
            ap=[list(t.ap[0])] + [list(d) for d in dims],
        )

    groups = [(t, min(2, NT - t)) for t in range(0, NT, 2)]  # (base, size)

    with tile.TileContext(nc) as tc:
        with (
            tc.tile_pool(name="singles", bufs=1) as singles,
            tc.tile_pool(name="xload", bufs=NT) as xload,
            tc.tile_pool(name="work", bufs=2) as work,
            tc.tile_pool(name="psum", bufs=2, space="PSUM") as psum,
        ):
            # DMA issue costs ~0.8us each on a sequencer: put the two
            # constant loads on the Scalar HWDGE queue (issues in parallel
            # with the Sync queue issuing x loads), and load x per tile-PAIR
            # to halve the issue count.
            wa_sb = singles.tile([CA, E3], BF16)
            nc.scalar.dma_start(out=wa_sb, in_=wa[:, :])
            cf_sb = singles.tile([P, 2 * E + P], F32)
            nc.scalar.dma_start(out=cf_sb, in_=cf[:, :])
            ga2_sb = cf_sb[0:2 * DA, 0:2 * E]
            id_sb = cf_sb[:, 2 * E:]

            xg_tiles = []
            xg_dmas = []
            for gi, (tg, g) in enumerate(groups):
                xg_sb = xload.tile([CA, K, 2 * P], BF16, name=f"xg_sb{gi}",
                                   tag="xg")
                xd = nc.sync.dma_start(
                    out=xg_sb[:, :, 0:g * P],
                    in_=xt[:, :, tg * P:(tg + g) * P])
                xg_tiles.append(xg_sb)
                xg_dmas.append(xd)

            out_sb = singles.tile([P, NT, E], F32)

            qkv_first = []
            for gi, (tg, g) in enumerate(groups):
                if gi >= 2:
                    # stagger loads ~2 groups ahead of compute so early tiles
                    # don't round-robin behind all the loads
                    tile.add_dep_helper(xg_dmas[gi].ins,
                                        qkv_first[gi - 2].ins,
                                        sync=True, reason="load stagger")
                qk_sb = work.tile([P, 2, K, E3], BF16)
                tmp = work.tile([P, 2, KK, D], BF16)
                for u in range(g):
                    t = tg + u
                    xt_sb = xg_tiles[gi][:, :, u * P:(u + 1) * P]

                    # ---- qkv: 6 matmuls (bf16 in, fp32 psum) ----
                    # [P, 8, 128] = exactly 2 PSUM banks so slots are
                    # bank-aligned (1.5-bank slots would share a bank)
                    qkv_ps = psum.tile([P, 8, 128], F32, tag="qkv_ps", bufs=2,
                                       name="qkv_ps")
                    for i in range(K):
                        mm = nc.tensor.matmul(
                            out=qkv_ps[:, i, 0:E3],
                            lhsT=xt_sb[:, i, :],
                            rhs=wa_sb[:, :],
                            start=True,
                            stop=True,
                        )
                        if i == 0 and u == 0:
                            qkv_first.append(mm)

                    # ---- evacuate q,k,v to SBUF as bf16 on ScalarE ----
                    nc.scalar.copy(out=qk_sb[:, u], in_=qkv_ps[:, 0:K, 0:E3])

                    # ---- scores tmp[i,j,(h,c)] = q[i,(hc)] * k[j,(hc)] ----
                    # (TensorTensor allows at most 3 free AP dims: (h,c) is
                    # kept merged, and the i/j broadcasts force per-tile muls)
                    off = u * K * E3
                    q_ap = apv(qk_sb, [[E3, K], [0, K], [1, D]], off)
                    k_ap = apv(qk_sb, [[0, K], [E3, K], [1, D]], off + D)
                    tm_out = apv(tmp, [[D, KK], [1, D]], u * KK * D)
                    nc.vector.tensor_mul(tm_out, q_ap, k_ap)

                # ---- batched over the pair from here on ----
                gKK = g * KK
                # scores[(t,i,j), h] = sum_c tmp via add tree: tensor_reduce
                # costs input-elems (2304/pair @1x) vs 1152+576+576 for the
                # tree (TT cost follows output elems)
                s1 = work.tile([P, 2, KK, H, 4], BF16)
                a_ap = apv(tmp, [[D, gKK], [HD, H], [1, 4]])
                b_ap = apv(tmp, [[D, gKK], [HD, H], [1, 4]], 4)
                o_ap = apv(s1, [[16, gKK], [4, H], [1, 4]])
                nc.vector.tensor_add(o_ap, a_ap, b_ap)
                s2 = work.tile([P, 2, KK, H, 2], BF16)
                a_ap = apv(s1, [[16, gKK], [4, H], [1, 2]])
                b_ap = apv(s1, [[16, gKK], [4, H], [1, 2]], 2)
                o_ap = apv(s2, [[8, gKK], [2, H], [1, 2]])
                nc.vector.tensor_add(o_ap, a_ap, b_ap)
                scores = work.tile([P, 2, KK, H], F32)
                a_ap = apv(s2, [[8, gKK], [2, H]])
                b_ap = apv(s2, [[8, gKK], [2, H]], 1)
                o_ap = apv(scores, [[H, gKK], [1, H]])
                nc.vector.tensor_add(o_ap, a_ap, b_ap)

                # ---- softmax over j (scores pre-scaled, |s|<1.5) ----
                es = work.tile([P, 2, K, K, H], BF16)
                nc.scalar.activation(out=es[:, 0:g], in_=scores[:, 0:g],
                                     func=AF.Exp)
                zs = work.tile([P, 2, K, H], F32)
                es_jred = apv(es, [[K * H, g * K], [1, H], [H, K]])  # [(t,i),h,j]
                zs_o = apv(zs, [[1, g * K * H]])
                nc.vector.reduce_sum(out=zs_o, in_=es_jred, axis=AX.X)
                rs = work.tile([P, 2, K, H], F32)
                nc.vector.reciprocal(rs[:, 0:g], zs[:, 0:g])
                # attn[(t,i),j,h] = es * rs
                attn = work.tile([P, 2, K, K, H], BF16)
                es_ap = apv(es, [[K * H, g * K], [H, K], [1, H]])
                rs_ap = apv(rs, [[H, g * K], [0, K], [1, H]])
                at_o = apv(attn, [[K * H, g * K], [H, K], [1, H]])
                nc.vector.tensor_mul(at_o, es_ap, rs_ap)

                # ---- wbar[t,(j,h)] = sum_i attn[t,i,(jh)] ----
                wbar = work.tile([P, 2, K, H], F32)
                at_ap = apv(attn, [[K * K * H, g], [1, K * H], [K * H, K]])
                wb_o = apv(wbar, [[K * H, g], [1, K * H]])
                nc.vector.reduce_sum(out=wb_o, in_=at_ap, axis=AX.X)

                # ---- ct[(t,j),h,c] = wbar[(t,jh)] * v[t,j,(hc)] ----
                ct = work.tile([P, 2, K, H, HD], BF16)
                wb_ap = apv(wbar, [[1, g * K * H], [0, HD]])
                v_ap = apv(qk_sb, [[K * E3, g], [E3, K], [1, D]], 2 * D)
                ct_o = apv(ct, [[1, g * K * D]])
                nc.vector.tensor_mul(ct_o, wb_ap, v_ap)

                # ---- ctxbar[t,(h,c)] = sum_j ct; col D set to 1 ----
                cb = work.tile([P, 2, DA], F32)
                ct_ap = apv(ct, [[K * D, g], [1, D], [D, K]])  # [t,(hc),j]
                cb_o = apv(cb, [[DA, g], [1, D]])
                nc.vector.reduce_sum(out=cb_o, in_=ct_ap, axis=AX.X)
                nc.vector.memset(cb[:, 0:g, D:DA], 1.0)

                # ---- logits: one transpose + one block-diag matmul ----
                ctT_ps = psum.tile([2 * DA, P], F32, bufs=2, name="ctT_ps")
                nc.tensor.transpose(ctT_ps[0:g * DA, :],
                                    cb[:, 0:g, :], id_sb)
                ctT_sb = work.tile([2 * DA, P], F32)
                nc.scalar.copy(out=ctT_sb[0:g * DA], in_=ctT_ps[0:g * DA])

                log_ps = psum.tile([P, 2 * E], F32, bufs=2, name="log_ps")
                nc.tensor.matmul(
                    out=log_ps[:, 0:g * E],
                    lhsT=ctT_sb[0:g * DA, :],
                    rhs=ga2_sb[0:g * DA, 0:g * E],
                    start=True, stop=True,
                )

                # ---- final softmax over 8 experts (batched) ----
                el = work.tile([P, 2, E], F32)
                nc.scalar.activation(out=el[:, 0:g], in_=log_ps[:, 0:g * E],
                                     func=AF.Exp)
                zf = work.tile([P, 2], F32)
                nc.vector.reduce_sum(out=zf[:, 0:g], in_=el[:, 0:g],
                                     axis=AX.X)
                rf = work.tile([P, 2], F32)
                nc.vector.reciprocal(rf[:, 0:g], zf[:, 0:g])
                rf_ap = apv(rf, [[1, g], [0, E]])
                nc.vector.tensor_mul(out_sb[:, tg:tg + g, :],
                                     el[:, 0:g], rf_ap)

            # single output store on the software-DGE path (one sync wait)
            nc.gpsimd.dma_start(out=out[:, :, :], in_=out_sb)

    nc.finalize()
    return nc


_NC = None


def _get_module():
    global _NC
    if _NC is None:
        _NC = _build_module()
    return _NC


def _host_prep(x, proj_w, proj_b, in_proj_w, in_proj_b, out_w, out_b, fc_w, fc_b):
    scale = np.float32(1.0 / np.sqrt(HD))
    w_eff = (in_proj_w @ proj_w).astype(np.float32)          # [96, 64]
    b_eff = (in_proj_w @ proj_b + in_proj_b).astype(np.float32)
    w_eff[0:D] *= scale
    b_eff[0:D] *= scale
    wa = np.concatenate([w_eff.T, b_eff[None, :]], axis=0)   # [65, 96]
    wa = np.ascontiguousarray(wa).astype(NP_BF16)

    g = (fc_w @ out_w / np.float32(K)).astype(np.float32)    # [8, 32]
    g_b = (fc_w @ out_b + fc_b).astype(np.float32)
    ga = np.concatenate([g.T, g_b[None, :]], axis=0)         # [33, 8]

    cf = np.zeros((P, 2 * E + P), dtype=np.float32)
    cf[0:DA, 0:E] = ga
    cf[DA:2 * DA, E:2 * E] = ga                               # block-diag
    cf[:, 2 * E:] = np.eye(P, dtype=np.float32)

    # x: [B, T, N, C] -> last K steps -> per-core [65, K, S_PAD] feature-major
    xk = x[:, T - K:, :, :]                                  # [B, K, N, C]
    in_maps = []
    for core in range(NCORES):
        xc = xk[core * B_SH:(core + 1) * B_SH]               # [8, K, N, C]
        # -> [C, K, b, N] -> [C, K, S]
        xc = np.transpose(xc, (3, 1, 0, 2)).reshape(C, K, S)
        xtc = np.ones((CA, K, S_PAD), dtype=NP_BF16)
        xtc[0:C, :, 0:S] = xc.astype(NP_BF16)
        xtc[0:C, :, S:] = 0
        in_maps.append({"xt": xtc, "wa": wa, "cf": cf})
    return in_maps


def kernel(x, proj_w, proj_b, in_proj_w, in_proj_b, out_w, out_b, fc_w, fc_b,
           _trace=False):
    in_maps = _host_prep(np.asarray(x, dtype=np.float32),
                         np.asarray(proj_w, dtype=np.float32),
                         np.asarray(proj_b, dtype=np.float32),
                         np.asarray(in_proj_w, dtype=np.float32),
                         np.asarray(in_proj_b, dtype=np.float32),
                         np.asarray(out_w, dtype=np.float32),
                         np.asarray(out_b, dtype=np.float32),
                         np.asarray(fc_w, dtype=np.float32),
                         np.asarray(fc_b, dtype=np.float32))
    nc = _get_module()
    res = run_bass_kernel_spmd(nc, in_maps, core_ids=list(range(NCORES)),
                               trace=_trace)
    outs = []
    for core in range(NCORES):
        oc = res.results[core]["out"]                        # [P, NT, E]
        oc = oc.transpose(1, 0, 2).reshape(S_PAD, E)[:S]
        oc = oc.reshape(B_SH, NTOK, E)
        outs.append(oc)
    full = np.concatenate(outs, axis=0)                      # [64, 207, 8]
    if _trace:
        kernel._last_exec_time_ns = res.exec_time_ns
        kernel._last_profile = res.profile_json
    return full.astype(np.float32)
